# revision 33
# baseline (speedup 1.0000x reference)
"""Trainium2 Bass kernel for nn_CausalSelfAttention (sparse windowed doc-masked
attention).

Sharding: tensor-parallel over heads - 8 heads onto 8 NeuronCores, one head per
core. Each core computes its head's QKV projection (fp16 matmuls at full PE
rate), RMS-norm + RoPE + key offset, value-embedding gating, block-sparse
masked attention in transposed-score layout, attention-output gating and its
partial output projection (fp16 partials). Host sums the 8 partial [T, DIM]
outputs.

x is supplied pre-transposed (xT fp16) by the host so no PE transposes are
needed for the QKV contraction; q/k rope outputs are transposed on the DMA
engines (blocked dma_start_transpose), the key-offset shift is a pair of
SBUF->SBUF DMAs, and emission is software-pipelined across documents so the
tensor engine stays busy (and clocked at full p-state) end to end.
"""

import numpy as np
import ml_dtypes
from contextlib import ExitStack

T = 4096
DIM = 1024
H = 8
D = 128
ATTN_SCALE = 0.1
EPS = 1.1920929e-07
QT = 512
NB = T // 128      # 32 row blocks
NQ = T // QT       # 8 q-tiles
ND = 4             # docs per emission group (blocks per doc group = NB // ND)
DB = NB // ND      # 8 blocks per doc group
F16 = ml_dtypes.bfloat16  # placeholder, replaced below
F16 = np.float16


def _rope_factors():
    n = D // 4
    base = np.float32(1.0 / 1024.0)
    af = base ** np.linspace(0.0, 1.0, n, dtype=np.float32)
    af = np.repeat(af, 2)
    af = np.concatenate([af, np.zeros(D // 2, np.float32)])
    theta = np.arange(T, dtype=np.float32)[:, None] * af[None, :]
    f1 = np.cos(theta).astype(np.float32)
    f2 = np.sin(theta).astype(np.float32)
    f2[:, 1::2] *= -1.0
    return f1, f2


def _plan_attention(seqlens, bm):
    """Per q-tile chunk lists with 0/1 keep-masks (sT layout [k, q])."""
    t = np.arange(T)
    doc = np.searchsorted(seqlens, t, side="right") - 1
    doc_start = np.where(doc >= 0, seqlens[np.clip(doc, 0, len(seqlens) - 1)], 0)
    lo = np.maximum(np.maximum(t - bm, doc_start), 0)

    masks = {}          # pattern-bytes -> (mask_id, keep01 [128, QT])
    plan = []           # per q-tile: list of (kc, mid, col_lo, col_hi, sub_any)
    for j in range(NQ):
        q = np.arange(j * QT, (j + 1) * QT)
        lo_q = lo[q]
        entries = []
        for kc in range(NB):
            k = np.arange(kc * 128, kc * 128 + 128)
            M = (k[:, None] <= q[None, :]) & (k[:, None] >= lo_q[None, :])
            if not M.any():
                continue
            anyk = M.any(axis=0)
            q0 = int(np.argmax(anyk))          # first q column with any valid k
            if M[:, q0:].all():
                mid, c0, c1 = None, 0, 0
            else:
                key = M.tobytes()
                if key not in masks:
                    masks[key] = (len(masks), M.astype(np.float32))
                mid = masks[key][0]
                part = ~M[:, :].all(axis=0)
                part[:q0] = False
                c0 = int(np.argmax(part))
                c1 = int(QT - np.argmax(part[::-1]))
            sub_any = tuple(bool(M[:, s * 128:(s + 1) * 128].any()) for s in range(4))
            entries.append((kc, mid, q0, c0, c1, sub_any))
        plan.append(entries)
    n_masks = len(masks)
    if n_masks:
        arr = np.zeros((n_masks * 128, QT), np.float32)
        for _, (mid, m) in masks.items():
            arr[mid * 128:(mid + 1) * 128] = m
    else:
        arr = np.ones((128, QT), np.float32)
    return plan, arr, max(n_masks, 1)


def _build(plan, n_masks, key_offset):
    import concourse.bass as bass
    import concourse.bacc as bacc
    import concourse.mybir as mybir
    from concourse import tile

    dt = mybir.dt
    f32, f16, i32 = dt.float32, dt.float16, dt.int32
    OP = mybir.AluOpType
    AF = mybir.ActivationFunctionType

    nc = bacc.Bacc()
    xt_d = nc.dram_tensor("xT", [DIM, T], f16, kind="ExternalInput").ap()
    wq_d = nc.dram_tensor("wqT", [DIM, 386], f16, kind="ExternalInput").ap()
    vb_d = nc.dram_tensor("vebias", [128, 2 * NB], f16, kind="ExternalInput").ap()
    wo_d = nc.dram_tensor("woT", [D, DIM], f16, kind="ExternalInput").ap()
    fve_d = nc.dram_tensor("fve", [T, 384], f16, kind="ExternalInput").ap()
    msk_d = nc.dram_tensor("masks", [n_masks * 128, QT], f16, kind="ExternalInput").ap()
    on_d = nc.dram_tensor("ones", [128, 128], f16, kind="ExternalInput").ap()
    gw_d = nc.dram_tensor("gwrep", [12, 128], f16, kind="ExternalInput").ap()
    out_d = nc.dram_tensor("out", [T, DIM], f16, kind="ExternalOutput").ap()

    with ExitStack() as ctx:
        tc = ctx.enter_context(tile.TileContext(nc))
        consts = ctx.enter_context(tc.tile_pool(name="consts", bufs=1))
        state = ctx.enter_context(tc.tile_pool(name="state", bufs=1))

        wq = consts.tile([128, 8 * 386], f16)
        nc.sync.dma_start(
            wq[:].rearrange("p (c n) -> p c n", n=386),
            wq_d[:].rearrange("(c p) n -> p c n", p=128),
        )
        ones = consts.tile([128, 128], f16)
        wo = consts.tile([128, DIM], f16)
        vb = consts.tile([128, 2 * NB], f16)
        msk = consts.tile([128, n_masks * QT], f16)
        fve = state.tile([128, NB * 384], f16)

        def load_consts2():
            nc.sync.dma_start(ones[:], on_d[:])
            nc.sync.dma_start(wo[:], wo_d[:])
            nc.sync.dma_start(vb[:], vb_d[:])
            nc.sync.dma_start(
                msk[:].rearrange("p (c n) -> p c n", n=QT),
                msk_d[:].rearrange("(c p) n -> p c n", p=128),
            )

        def stage_fve(g):
            nc.sync.dma_start(
                fve[:, g * 8 * 384:(g + 1) * 8 * 384].rearrange("p (b n) -> p b n", n=384),
                fve_d[g * 1024:(g + 1) * 1024, :].rearrange("(b p) n -> p b n", p=128),
            )

        qkvg_sb = state.tile([128, NB * 386], f16)
        ro_q = state.tile([128, T], f16)
        ro_k = state.tile([128, T], f16)
        qT = state.tile([128, T], f16)
        kTt = state.tile([128, T], f16)
        kT = state.tile([128, T + 128], f16, name="kT") if key_offset else kTt
        yT = state.tile([128, T], f16)
        bn_q = state.tile([128, 2 * NB], f32)
        bn_k = state.tile([128, 2 * NB], f32)
        rs_q = state.tile([128, NB], f32)
        rs_k = state.tile([128, NB], f32)
        gates_sig = state.tile([128, 2 * NB], f32)
        recip = state.tile([128, NB], f32)
        sfin = state.tile([128, NB], f32)

        qg3 = qkvg_sb[:].rearrange("p (b c) -> p b c", c=386)
        fv3 = fve[:].rearrange("p (b c) -> p b c", c=384)
        gs3 = gates_sig[:].rearrange("p (b w) -> p b w", w=2)
        vb3 = vb[:].rearrange("p (b w) -> p b w", w=2)

        xpool = ctx.enter_context(tc.tile_pool(name="xg", bufs=2))
        scr = ctx.enter_context(tc.tile_pool(name="scr", bufs=6))
        ppool = ctx.enter_context(tc.tile_pool(name="pch", bufs=6))
        obpool = ctx.enter_context(tc.tile_pool(name="ob", bufs=2))
        qppool = ctx.enter_context(tc.tile_pool(name="qp", bufs=2, space="PSUM"))
        sppool = ctx.enter_context(tc.tile_pool(name="sp", bufs=3, space="PSUM"))
        ypool = ctx.enter_context(tc.tile_pool(name="yp", bufs=2, space="PSUM"))
        dpool = ctx.enter_context(tc.tile_pool(name="dp", bufs=1, space="PSUM"))

        xg_tiles = {}

        def stage_xdma(g, split=False):
            xg = xpool.tile([128, 8 * 1024], f16)
            halves = ((0, 256), (256, 1024)) if split else ((0, 512), (512, 1024))
            for lo, hi in halves:
                nc.sync.dma_start(
                    xg[:].rearrange("p (c t) -> p c t", t=1024)[:, :, lo:hi],
                    xt_d[:, g * 1024 + lo:g * 1024 + hi].rearrange("(c p) t -> p c t", p=128),
                )
            xg_tiles[g] = xg

        def stage_a_block(b):
            """QKV matmuls for block b + psum->sbuf copies + sumsq."""
            g, bl = b // DB, b % DB
            xg = xg_tiles[g]
            bc = slice(128 * b, 128 * (b + 1))
            qpt = qppool.tile([128, 386], f32, tag="qp", name="qpt")
            qp = qpt[:]
            for ci in range(8):
                nc.tensor.matmul(qp, lhsT=xg[:, 1024 * ci + 128 * bl:1024 * ci + 128 * (bl + 1)],
                                 rhs=wq[:, 386 * ci:386 * (ci + 1)],
                                 start=(ci == 0), stop=(ci == 7))
            nc.scalar.copy(qg3[:, b, :], qp)
            for (co, bnt) in ((0, bn_q), (128, bn_k)):
                bns = scr.tile([128, 6], f32, tag="bns")
                nc.vector.bn_stats(bns[:], qg3[:, b, co:co + 128])
                nc.vector.bn_aggr(bnt[:, 2 * b:2 * b + 2], bns[:])

        def stage_b_doc(d):
            """rsqrt + gates + v-embed gating + rope for blocks of doc-group d."""
            bs = slice(DB * d, DB * (d + 1))
            for bnt, rs in ((bn_q, rs_q), (bn_k, rs_k)):
                # E[x^2] = mean^2 + var; Quake rsqrt + 2 Newton steps
                bg = bnt[:].rearrange("p (b w) -> p b w", w=2)[:, bs, :]
                sqm = scr.tile([128, DB], f32, tag="sqm")
                nc.vector.tensor_tensor(sqm[:], bg[:, :, 0], bg[:, :, 0], op=OP.mult)
                m = scr.tile([128, DB], f32, tag="rsm")
                nc.vector.scalar_tensor_tensor(out=m[:], in0=bg[:, :, 1], scalar=EPS,
                                               in1=sqm[:], op0=OP.add, op1=OP.add)
                ii = scr.tile([128, DB], i32, tag="rsi")
                nc.vector.tensor_scalar(out=ii[:], in0=m[:].bitcast(i32), scalar1=1,
                                        scalar2=None, op0=OP.logical_shift_right)
                nc.vector.tensor_scalar(out=ii[:], in0=ii[:], scalar1=-1,
                                        scalar2=0x5F3759DF, op0=OP.mult, op1=OP.add)
                y = ii[:].bitcast(f32)
                tt = scr.tile([128, DB], f32, tag="rst")
                nc.vector.tensor_mul(tt[:], y, y)
                nc.vector.tensor_mul(tt[:], tt[:], m[:])
                nc.vector.tensor_scalar(out=tt[:], in0=tt[:], scalar1=-0.5,
                                        scalar2=1.5, op0=OP.mult, op1=OP.add)
                nc.vector.tensor_mul(y, y, tt[:])
                nc.vector.tensor_mul(tt[:], y, y)
                nc.vector.tensor_mul(tt[:], tt[:], m[:])
                nc.vector.tensor_scalar(out=tt[:], in0=tt[:], scalar1=-0.5,
                                        scalar2=1.5, op0=OP.mult, op1=OP.add)
                nc.vector.tensor_mul(rs[:, bs], y, tt[:])
            # sigmoid(x) = 1 / (1 + exp(-x)) using the Exp table + DVE recip
            gsl = slice(2 * DB * d, 2 * DB * (d + 1))
            nc.vector.tensor_tensor(qg3[:, bs, 384:386], qg3[:, bs, 384:386],
                                    vb3[:, bs, :], op=OP.add)
            ge = scr.tile([128, 2 * DB], f32, tag="ge")
            nc.scalar.activation(ge[:].rearrange("p (b w) -> p b w", w=2),
                                 qg3[:, bs, 384:386], AF.Exp, scale=-1.0)
            nc.vector.tensor_scalar(out=ge[:], in0=ge[:], scalar1=1.0,
                                    scalar2=None, op0=OP.add)
            nc.vector.reciprocal(gates_sig[:, gsl], ge[:])
            for b in range(DB * d, DB * (d + 1)):
                nc.vector.scalar_tensor_tensor(out=qg3[:, b, 256:384], in0=fv3[:, b, 256:384],
                                               scalar=gs3[:, b, 0:1], in1=qg3[:, b, 256:384],
                                               op0=OP.mult, op1=OP.add)
            # rope: two half-doc-wide sweeps per tensor, then per-block rms scale
            for half in range(2):
                blk = slice(DB * d + 4 * half, DB * d + 4 * half + 4)
                cols = slice(512 * (2 * d + half), 512 * (2 * d + half + 1))
                for (co, ro, rs) in ((0, ro_q, rs_q), (128, ro_k, rs_k)):
                    src = qg3[:, blk, co:co + 128]
                    srcf = src.rearrange("p b (n w) -> p b n w", w=2)[:, :, :, ::-1]
                    eng1 = nc.vector if d in (0, 2, 3) else nc.gpsimd
                    t1 = scr.tile([128, 512], f16, tag="t1")
                    eng1.tensor_tensor(t1[:].rearrange("p (b n) -> p b n", n=128),
                                       fv3[:, blk, 0:128], src, op=OP.mult)
                    t2 = scr.tile([128, 512], f16, tag="t2")
                    nc.vector.tensor_tensor(
                        t2[:].rearrange("p (b n w) -> p b n w", n=64, w=2),
                        fv3[:, blk, 128:256].rearrange("p b (n w) -> p b n w", w=2),
                        srcf, op=OP.mult)
                    eng1.tensor_tensor(ro[:, cols], t1[:], t2[:], op=OP.add)
                for b in range(DB * d + 4 * half, DB * d + 4 * half + 4):
                    bc = slice(128 * b, 128 * (b + 1))
                    nc.vector.tensor_tensor(ro_q[:, bc], ro_q[:, bc],
                                            rs_q[:, b:b + 1].broadcast_to((128, 128)), op=OP.mult)
                    nc.vector.tensor_tensor(ro_k[:, bc], ro_k[:, bc],
                                            rs_k[:, b:b + 1].broadcast_to((128, 128)), op=OP.mult)

        def stage_c_doc(d):
            """transpose q/k for doc-group d (DMA engines) + key-offset shift."""
            dc = slice(1024 * d, 1024 * (d + 1))
            nc.sync.dma_start_transpose(
                qT[:, dc].rearrange("p (b t) -> p b t", t=128), ro_q[:, dc])
            nc.sync.dma_start_transpose(
                kTt[:, dc].rearrange("p (b t) -> p b t", t=128), ro_k[:, dc])
            if key_offset:
                nc.sync.dma_start(kT[0:64, dc], kTt[0:64, dc])
                if d == 0:
                    nc.sync.dma_start(kT[64:128, 1:1024], kTt[64:128, 0:1023])
                    nc.sync.dma_start(kT[64:128, 0:1], kTt[64:128, 0:1])
                else:
                    nc.sync.dma_start(kT[64:128, 1024 * d:1024 * (d + 1)],
                                      kTt[64:128, 1024 * d - 1:1024 * (d + 1) - 1])

        def stage_d_attn(j):
            """attention for q-tile j -> yT."""
            qs = slice(QT * j, QT * (j + 1))
            entries = plan[j]
            y_ps = ypool.tile([128, QT], f32, tag="y")
            den_ps = dpool.tile([128, 8], f32, tag="den")
            den_pairs = [(kc, s)
                         for ii, (kc, _, q0e, _, _, sa) in enumerate(entries)
                         for s in range(4)
                         if sa[s] and (128 * (s + 1) > (0 if ii == 0 else q0e))]
            den_first, den_last = den_pairs[0], den_pairs[-1]

            def pv_and_den(ei, kc, q0, sub_any, p_sb):
                nc.tensor.matmul(y_ps[:, q0:QT], lhsT=qg3[:, kc, 256:384],
                                 rhs=p_sb[:, q0:QT],
                                 start=(ei == 0), stop=(ei == len(entries) - 1),
                                 skip_group_check=True)
                for s in range(4):
                    if sub_any[s] and 128 * (s + 1) > q0:
                        nc.tensor.matmul(den_ps[:, 2 * s:2 * s + 2],
                                         lhsT=p_sb[:, 128 * s:128 * (s + 1)],
                                         rhs=ones[:, 0:2],
                                         start=((kc, s) == den_first),
                                         stop=((kc, s) == den_last),
                                         skip_group_check=True)

            pend_pv = []
            for ei, (kc, mid, q0, c0, c1, sub_any) in enumerate(entries):
                if ei == 0:
                    q0 = 0  # first entry must zero the whole psum bank
                kcc = slice(128 * kc, 128 * (kc + 1))
                sp = sppool.tile([128, QT], f32, tag="s")
                nc.tensor.matmul(sp[:, q0:QT], lhsT=kT[:, kcc],
                                 rhs=qT[:, QT * j + q0:QT * (j + 1)],
                                 start=True, stop=True)
                p_sb = ppool.tile([128, QT], f16, tag="p")
                nc.scalar.activation(p_sb[:, q0:QT], sp[:, q0:QT], AF.Exp,
                                     scale=ATTN_SCALE)
                if mid is not None:
                    nc.gpsimd.tensor_tensor(p_sb[:, c0:c1], p_sb[:, c0:c1],
                                            msk[:, QT * mid + c0:QT * mid + c1],
                                            op=OP.mult)
                pend_pv.append((ei, kc, q0, sub_any, p_sb))
                if len(pend_pv) >= 3:
                    pv_and_den(*pend_pv.pop(0))
            while pend_pv:
                pv_and_den(*pend_pv.pop(0))
            den3 = den_ps[:].rearrange("p (s w) -> p s w", w=2)
            nc.vector.reciprocal(recip[:, 4 * j:4 * j + 4], den3[:, :, 0])
            nc.vector.tensor_tensor(sfin[:, 4 * j:4 * j + 4], recip[:, 4 * j:4 * j + 4],
                                    gs3[:, 4 * j:4 * j + 4, 1], op=OP.mult)
            nc.vector.tensor_copy(yT[:, qs], y_ps[:])

        def stage_d_oproj(j):
            """output projection + store for q-tile j."""
            qs = slice(QT * j, QT * (j + 1))
            ob = obpool.tile([128, 4 * DIM], f16, tag="ob")
            for s in range(4):
                b = 4 * j + s
                bc = slice(128 * b, 128 * (b + 1))
                for hh in range(2):
                    op_ps = sppool.tile([128, 512], f32, tag="s", name="op_ps")
                    nc.tensor.matmul(op_ps[:], lhsT=yT[:, bc],
                                     rhs=wo[:, 512 * hh:512 * (hh + 1)],
                                     start=True, stop=True)
                    dst = ob[:, 1024 * s + 512 * hh:1024 * s + 512 * (hh + 1)]
                    if (2 * s + hh) % 2 == 0:
                        nc.scalar.mul(dst, op_ps[:], sfin[:, b:b + 1])
                    else:
                        nc.vector.tensor_scalar_mul(dst, op_ps[:], sfin[:, b:b + 1])
                if s % 2 == 1:
                    rows = slice(QT * j + 128 * (s - 1), QT * j + 128 * (s + 1))
                    nc.sync.dma_start(
                        out_d[rows, :].rearrange("(s2 p) n -> p s2 n", p=128),
                        ob[:, 1024 * (s - 1):1024 * (s + 1)].rearrange(
                            "p (s2 n) -> p s2 n", n=DIM),
                    )

        # ---- emission schedule: attention interleaved with later-doc QKV ----
        stage_xdma(0, split=True)
        stage_xdma(1)
        stage_fve(0)
        stage_fve(1)
        load_consts2()
        stage_xdma(2)
        stage_fve(2)
        stage_xdma(3)
        stage_fve(3)
        for b in range(0, DB):
            stage_a_block(b)
        stage_b_doc(0)
        stage_c_doc(0)
        for b in range(DB, DB + 4):
            stage_a_block(b)
        stage_d_attn(0)
        for b in range(DB + 4, 2 * DB):
            stage_a_block(b)
        stage_b_doc(1)
        stage_c_doc(1)
        for b in range(2 * DB, 2 * DB + 4):
            stage_a_block(b)
        stage_d_attn(1)
        stage_d_oproj(0)
        for b in range(2 * DB + 4, 3 * DB):
            stage_a_block(b)
        stage_b_doc(2)
        stage_c_doc(2)
        for b in range(3 * DB, 3 * DB + 4):
            stage_a_block(b)
        stage_d_attn(2)
        stage_d_oproj(1)
        for b in range(3 * DB + 4, 4 * DB):
            stage_a_block(b)
        stage_b_doc(3)
        stage_c_doc(3)
        stage_d_attn(3)
        stage_d_oproj(2)
        stage_d_attn(4)
        stage_d_oproj(3)
        stage_d_attn(5)
        stage_d_oproj(4)
        stage_d_attn(6)
        stage_d_oproj(5)
        stage_d_attn(7)
        stage_d_oproj(6)
        stage_d_oproj(7)
    nc.finalize()
    return nc


_CACHE = {}


def _get_program(seqlens, bm, key_offset):
    key = (seqlens.tobytes(), int(bm), int(key_offset))
    if key not in _CACHE:
        plan, mask_arr, n_masks = _plan_attention(seqlens, bm)
        nc = _build(plan, n_masks, key_offset)
        _CACHE[key] = (nc, mask_arr, n_masks)
    return _CACHE[key]


def _make_inmaps(x, ve, qkvo_w, sa_lambdas, attn_gate_w, ve_gate_w, mask_arr):
    f1, f2 = _rope_factors()
    ones = np.ones((128, 128), F16)
    x2 = x.reshape(T, DIM)
    xT = np.ascontiguousarray(x2.T).astype(F16)
    wqkv = (sa_lambdas[0] * qkvo_w[:3 * DIM]).astype(np.float32)
    wo = (sa_lambdas[1] * qkvo_w[3 * DIM:]).astype(np.float32)
    msk16 = mask_arr.astype(F16)
    in_maps = []
    for h in range(H):
        hs = slice(h * D, (h + 1) * D)
        w_h = np.concatenate([wqkv[0 * DIM:][hs], wqkv[1 * DIM:][hs], wqkv[2 * DIM:][hs]], axis=0)
        wq386 = np.zeros((DIM, 386), np.float32)
        wq386[:, :384] = w_h.T
        wq386[:6, 384] = ve_gate_w[h, :6]
        wq386[:12, 385] = attn_gate_w[h, :12]
        vebias = np.zeros((128, 2 * NB), np.float32)
        vebias[:, 0::2] = (ve[:, :6] @ ve_gate_w[h, 6:12]).reshape(NB, 128).T
        fve = np.concatenate([f1, f2, 2.0 * ve[:, hs]], axis=1)
        in_maps.append({
            "xT": xT,
            "wqT": wq386.astype(F16),
            "vebias": vebias.astype(F16),
            "woT": np.ascontiguousarray(wo[:, hs].T).astype(F16),
            "fve": fve.astype(F16),
            "masks": msk16,
            "ones": ones,
            "gwrep": np.repeat(attn_gate_w[h, :12].astype(np.float32)[:, None], 128, 1).astype(F16),
        })
    return in_maps


def _run(inputs, trace=False):
    from concourse.bass_utils import run_bass_kernel_spmd

    x = np.asarray(inputs["x"], np.float32)
    ve = np.asarray(inputs["ve"], np.float32)
    qkvo_w = np.asarray(inputs["qkvo_w"], np.float32)
    sa_lambdas = np.asarray(inputs["sa_lambdas"], np.float32)
    attn_gate_w = np.asarray(inputs["attn_gate_w"], np.float32)
    ve_gate_w = np.asarray(inputs["ve_gate_w"], np.float32)
    seqlens = np.asarray(inputs["seqlens"])
    bm = int(np.asarray(inputs["bm_size"]))
    key_offset = int(np.asarray(inputs["key_offset"]))

    nc, mask_arr, _ = _get_program(seqlens, bm, key_offset)
    in_maps = _make_inmaps(x, ve, qkvo_w, sa_lambdas, attn_gate_w, ve_gate_w, mask_arr)
    res = run_bass_kernel_spmd(nc, in_maps, core_ids=list(range(H)), trace=trace)
    out = np.zeros((T, DIM), np.float32)
    for r in res.results:
        out += np.asarray(r["out"]).astype(np.float32)
    return out.reshape(1, T, DIM), res


def kernel(**inputs) -> np.ndarray:
    out, _ = _run(inputs, trace=False)
    return out


# revision 35
# speedup vs baseline: 1.0308x; 1.0308x over previous
"""Trainium2 Bass kernel for nn_CausalSelfAttention (sparse windowed doc-masked
attention).

Sharding: tensor-parallel over heads - 8 heads onto 8 NeuronCores, one head per
core. Each core computes its head's QKV projection (fp16 matmuls at full PE
rate), RMS-norm + RoPE + key offset, value-embedding gating, block-sparse
masked attention in transposed-score layout, attention-output gating and its
partial output projection (fp16 partials). Host sums the 8 partial [T, DIM]
outputs.

x is supplied pre-transposed (xT fp16) by the host so no PE transposes are
needed for the QKV contraction; q/k rope outputs are transposed on the DMA
engines (blocked dma_start_transpose), the key-offset shift is a pair of
SBUF->SBUF DMAs, and emission is software-pipelined across documents so the
tensor engine stays busy (and clocked at full p-state) end to end.
"""

import numpy as np
import ml_dtypes
from contextlib import ExitStack

T = 4096
DIM = 1024
H = 8
D = 128
ATTN_SCALE = 0.1
EPS = 1.1920929e-07
QT = 512
NB = T // 128      # 32 row blocks
NQ = T // QT       # 8 q-tiles
ND = 4             # docs per emission group (blocks per doc group = NB // ND)
DB = NB // ND      # 8 blocks per doc group
F16 = ml_dtypes.bfloat16  # placeholder, replaced below
F16 = np.float16


def _rope_factors():
    n = D // 4
    base = np.float32(1.0 / 1024.0)
    af = base ** np.linspace(0.0, 1.0, n, dtype=np.float32)
    af = np.repeat(af, 2)
    af = np.concatenate([af, np.zeros(D // 2, np.float32)])
    theta = np.arange(T, dtype=np.float32)[:, None] * af[None, :]
    f1 = np.cos(theta).astype(np.float32)
    f2 = np.sin(theta).astype(np.float32)
    f2[:, 1::2] *= -1.0
    return f1, f2


def _plan_attention(seqlens, bm):
    """Per q-tile chunk lists with 0/1 keep-masks (sT layout [k, q])."""
    t = np.arange(T)
    doc = np.searchsorted(seqlens, t, side="right") - 1
    doc_start = np.where(doc >= 0, seqlens[np.clip(doc, 0, len(seqlens) - 1)], 0)
    lo = np.maximum(np.maximum(t - bm, doc_start), 0)

    masks = {}          # pattern-bytes -> (mask_id, keep01 [128, QT])
    plan = []           # per q-tile: list of (kc, mid, col_lo, col_hi, sub_any)
    for j in range(NQ):
        q = np.arange(j * QT, (j + 1) * QT)
        lo_q = lo[q]
        entries = []
        for kc in range(NB):
            k = np.arange(kc * 128, kc * 128 + 128)
            M = (k[:, None] <= q[None, :]) & (k[:, None] >= lo_q[None, :])
            if not M.any():
                continue
            anyk = M.any(axis=0)
            q0 = int(np.argmax(anyk))          # first q column with any valid k
            if M[:, q0:].all():
                mid, c0, c1 = None, 0, 0
            else:
                key = M.tobytes()
                if key not in masks:
                    masks[key] = (len(masks), M.astype(np.float32))
                mid = masks[key][0]
                part = ~M[:, :].all(axis=0)
                part[:q0] = False
                c0 = int(np.argmax(part))
                c1 = int(QT - np.argmax(part[::-1]))
            sub_any = tuple(bool(M[:, s * 128:(s + 1) * 128].any()) for s in range(4))
            entries.append((kc, mid, q0, c0, c1, sub_any))
        plan.append(entries)
    n_masks = len(masks)
    if n_masks:
        arr = np.zeros((n_masks * 128, QT), np.float32)
        for _, (mid, m) in masks.items():
            arr[mid * 128:(mid + 1) * 128] = m
    else:
        arr = np.ones((128, QT), np.float32)
    return plan, arr, max(n_masks, 1)


def _build(plan, n_masks, key_offset):
    import concourse.bass as bass
    import concourse.bacc as bacc
    import concourse.mybir as mybir
    from concourse import tile

    dt = mybir.dt
    f32, f16, i32 = dt.float32, dt.float16, dt.int32
    OP = mybir.AluOpType
    AF = mybir.ActivationFunctionType

    nc = bacc.Bacc()
    xt_d = nc.dram_tensor("xT", [DIM, T], f16, kind="ExternalInput").ap()
    wq_d = nc.dram_tensor("wqT", [DIM, 386], f16, kind="ExternalInput").ap()
    vb_d = nc.dram_tensor("vebias", [128, 2 * NB], f16, kind="ExternalInput").ap()
    wo_d = nc.dram_tensor("woT", [D, DIM], f16, kind="ExternalInput").ap()
    fve_d = nc.dram_tensor("fve", [T, 384], f16, kind="ExternalInput").ap()
    msk_d = nc.dram_tensor("masks", [n_masks * 128, QT], f16, kind="ExternalInput").ap()
    on_d = nc.dram_tensor("ones", [128, 128], f16, kind="ExternalInput").ap()
    gw_d = nc.dram_tensor("gwrep", [12, 128], f16, kind="ExternalInput").ap()
    out_d = nc.dram_tensor("out", [T, DIM], f16, kind="ExternalOutput").ap()

    with ExitStack() as ctx:
        tc = ctx.enter_context(tile.TileContext(nc))
        consts = ctx.enter_context(tc.tile_pool(name="consts", bufs=1))
        state = ctx.enter_context(tc.tile_pool(name="state", bufs=1))

        wq = consts.tile([128, 8 * 386], f16)
        nc.sync.dma_start(
            wq[:].rearrange("p (c n) -> p c n", n=386),
            wq_d[:].rearrange("(c p) n -> p c n", p=128),
        )
        ones = consts.tile([128, 128], f16)
        wo = consts.tile([128, DIM], f16)
        vb = consts.tile([128, 2 * NB], f16)
        msk = consts.tile([128, n_masks * QT], f16)
        fve = state.tile([128, NB * 384], f16)

        def load_consts2():
            nc.sync.dma_start(ones[:], on_d[:])
            nc.sync.dma_start(wo[:], wo_d[:])
            nc.sync.dma_start(vb[:], vb_d[:])
            nc.sync.dma_start(
                msk[:].rearrange("p (c n) -> p c n", n=QT),
                msk_d[:].rearrange("(c p) n -> p c n", p=128),
            )

        def stage_fve(g):
            nc.sync.dma_start(
                fve[:, g * 8 * 384:(g + 1) * 8 * 384].rearrange("p (b n) -> p b n", n=384),
                fve_d[g * 1024:(g + 1) * 1024, :].rearrange("(b p) n -> p b n", p=128),
            )

        qkvg_sb = state.tile([128, NB * 386], f16)
        ro_q = state.tile([128, T], f16)
        ro_k = state.tile([128, T], f16)
        qT = state.tile([128, T], f16)
        kTt = state.tile([128, T], f16)
        kT = state.tile([128, T + 128], f16, name="kT") if key_offset else kTt
        yT = state.tile([128, T], f16)
        bn_q = state.tile([128, 2 * NB], f32)
        bn_k = state.tile([128, 2 * NB], f32)
        rs_q = state.tile([128, NB], f32)
        rs_k = state.tile([128, NB], f32)
        gates_sig = state.tile([128, 2 * NB], f32)
        recip = state.tile([128, NB], f32)
        sfin = state.tile([128, NB], f32)

        qg3 = qkvg_sb[:].rearrange("p (b c) -> p b c", c=386)
        fv3 = fve[:].rearrange("p (b c) -> p b c", c=384)
        gs3 = gates_sig[:].rearrange("p (b w) -> p b w", w=2)
        vb3 = vb[:].rearrange("p (b w) -> p b w", w=2)

        xpool = ctx.enter_context(tc.tile_pool(name="xg", bufs=2))
        scr = ctx.enter_context(tc.tile_pool(name="scr", bufs=6))
        ppool = ctx.enter_context(tc.tile_pool(name="pch", bufs=6))
        obpool = ctx.enter_context(tc.tile_pool(name="ob", bufs=2))
        qppool = ctx.enter_context(tc.tile_pool(name="qp", bufs=2, space="PSUM"))
        sppool = ctx.enter_context(tc.tile_pool(name="sp", bufs=4, space="PSUM"))
        ypool = ctx.enter_context(tc.tile_pool(name="yp", bufs=1, space="PSUM"))
        dpool = ctx.enter_context(tc.tile_pool(name="dp", bufs=1, space="PSUM"))

        xg_tiles = {}

        def stage_xdma(g, split=False):
            xg = xpool.tile([128, 8 * 1024], f16)
            halves = ((0, 256), (256, 1024)) if split else ((0, 512), (512, 1024))
            for lo, hi in halves:
                nc.sync.dma_start(
                    xg[:].rearrange("p (c t) -> p c t", t=1024)[:, :, lo:hi],
                    xt_d[:, g * 1024 + lo:g * 1024 + hi].rearrange("(c p) t -> p c t", p=128),
                )
            xg_tiles[g] = xg

        def stage_a_block(b):
            """QKV matmuls for block b + psum->sbuf copies + sumsq."""
            g, bl = b // DB, b % DB
            xg = xg_tiles[g]
            bc = slice(128 * b, 128 * (b + 1))
            qpt = qppool.tile([128, 386], f32, tag="qp", name="qpt")
            qp = qpt[:]
            for ci in range(8):
                nc.tensor.matmul(qp, lhsT=xg[:, 1024 * ci + 128 * bl:1024 * ci + 128 * (bl + 1)],
                                 rhs=wq[:, 386 * ci:386 * (ci + 1)],
                                 start=(ci == 0), stop=(ci == 7))
            nc.scalar.copy(qg3[:, b, :], qp)
            for (co, bnt) in ((0, bn_q), (128, bn_k)):
                bns = scr.tile([128, 6], f32, tag="bns")
                nc.vector.bn_stats(bns[:], qg3[:, b, co:co + 128])
                nc.vector.bn_aggr(bnt[:, 2 * b:2 * b + 2], bns[:])

        def stage_b_doc(d):
            """rsqrt + gates + v-embed gating + rope for blocks of doc-group d."""
            bs = slice(DB * d, DB * (d + 1))
            for bnt, rs in ((bn_q, rs_q), (bn_k, rs_k)):
                # E[x^2] = mean^2 + var; Quake rsqrt + 2 Newton steps
                bg = bnt[:].rearrange("p (b w) -> p b w", w=2)[:, bs, :]
                sqm = scr.tile([128, DB], f32, tag="sqm")
                nc.vector.tensor_tensor(sqm[:], bg[:, :, 0], bg[:, :, 0], op=OP.mult)
                m = scr.tile([128, DB], f32, tag="rsm")
                nc.vector.scalar_tensor_tensor(out=m[:], in0=bg[:, :, 1], scalar=EPS,
                                               in1=sqm[:], op0=OP.add, op1=OP.add)
                ii = scr.tile([128, DB], i32, tag="rsi")
                nc.vector.tensor_scalar(out=ii[:], in0=m[:].bitcast(i32), scalar1=1,
                                        scalar2=None, op0=OP.logical_shift_right)
                nc.vector.tensor_scalar(out=ii[:], in0=ii[:], scalar1=-1,
                                        scalar2=0x5F3759DF, op0=OP.mult, op1=OP.add)
                y = ii[:].bitcast(f32)
                tt = scr.tile([128, DB], f32, tag="rst")
                nc.vector.tensor_mul(tt[:], y, y)
                nc.vector.tensor_mul(tt[:], tt[:], m[:])
                nc.vector.tensor_scalar(out=tt[:], in0=tt[:], scalar1=-0.5,
                                        scalar2=1.5, op0=OP.mult, op1=OP.add)
                nc.vector.tensor_mul(y, y, tt[:])
                nc.vector.tensor_mul(tt[:], y, y)
                nc.vector.tensor_mul(tt[:], tt[:], m[:])
                nc.vector.tensor_scalar(out=tt[:], in0=tt[:], scalar1=-0.5,
                                        scalar2=1.5, op0=OP.mult, op1=OP.add)
                nc.vector.tensor_mul(rs[:, bs], y, tt[:])
            # sigmoid(x) = 1 / (1 + exp(-x)) using the Exp table + DVE recip
            gsl = slice(2 * DB * d, 2 * DB * (d + 1))
            nc.vector.tensor_tensor(qg3[:, bs, 384:386], qg3[:, bs, 384:386],
                                    vb3[:, bs, :], op=OP.add)
            ge = scr.tile([128, 2 * DB], f32, tag="ge")
            nc.scalar.activation(ge[:].rearrange("p (b w) -> p b w", w=2),
                                 qg3[:, bs, 384:386], AF.Exp, scale=-1.0)
            nc.vector.tensor_scalar(out=ge[:], in0=ge[:], scalar1=1.0,
                                    scalar2=None, op0=OP.add)
            nc.vector.reciprocal(gates_sig[:, gsl], ge[:])
            for b in range(DB * d, DB * (d + 1)):
                nc.vector.scalar_tensor_tensor(out=qg3[:, b, 256:384], in0=fv3[:, b, 256:384],
                                               scalar=gs3[:, b, 0:1], in1=qg3[:, b, 256:384],
                                               op0=OP.mult, op1=OP.add)
            # rope: two half-doc-wide sweeps per tensor, then per-block rms scale
            for half in range(2):
                blk = slice(DB * d + 4 * half, DB * d + 4 * half + 4)
                cols = slice(512 * (2 * d + half), 512 * (2 * d + half + 1))
                for (co, ro, rs) in ((0, ro_q, rs_q), (128, ro_k, rs_k)):
                    src = qg3[:, blk, co:co + 128]
                    srcf = src.rearrange("p b (n w) -> p b n w", w=2)[:, :, :, ::-1]
                    eng1 = nc.vector if d in (0, 3) else nc.gpsimd
                    t1 = scr.tile([128, 512], f16, tag="t1")
                    eng1.tensor_tensor(t1[:].rearrange("p (b n) -> p b n", n=128),
                                       fv3[:, blk, 0:128], src, op=OP.mult)
                    t2 = scr.tile([128, 512], f16, tag="t2")
                    nc.vector.tensor_tensor(
                        t2[:].rearrange("p (b n w) -> p b n w", n=64, w=2),
                        fv3[:, blk, 128:256].rearrange("p b (n w) -> p b n w", w=2),
                        srcf, op=OP.mult)
                    eng1.tensor_tensor(ro[:, cols], t1[:], t2[:], op=OP.add)
                for b in range(DB * d + 4 * half, DB * d + 4 * half + 4):
                    bc = slice(128 * b, 128 * (b + 1))
                    nc.vector.tensor_tensor(ro_q[:, bc], ro_q[:, bc],
                                            rs_q[:, b:b + 1].broadcast_to((128, 128)), op=OP.mult)
                    nc.vector.tensor_tensor(ro_k[:, bc], ro_k[:, bc],
                                            rs_k[:, b:b + 1].broadcast_to((128, 128)), op=OP.mult)

        def stage_c_doc(d):
            """transpose q/k for doc-group d (DMA engines) + key-offset shift."""
            dc = slice(1024 * d, 1024 * (d + 1))
            nc.sync.dma_start_transpose(
                qT[:, dc].rearrange("p (b t) -> p b t", t=128), ro_q[:, dc])
            nc.sync.dma_start_transpose(
                kTt[:, dc].rearrange("p (b t) -> p b t", t=128), ro_k[:, dc])
            if key_offset:
                nc.sync.dma_start(kT[0:64, dc], kTt[0:64, dc])
                if d == 0:
                    nc.sync.dma_start(kT[64:128, 1:1024], kTt[64:128, 0:1023])
                    nc.sync.dma_start(kT[64:128, 0:1], kTt[64:128, 0:1])
                else:
                    nc.sync.dma_start(kT[64:128, 1024 * d:1024 * (d + 1)],
                                      kTt[64:128, 1024 * d - 1:1024 * (d + 1) - 1])

        def stage_d_attn(j):
            """attention for q-tile j -> yT."""
            qs = slice(QT * j, QT * (j + 1))
            entries = plan[j]
            y_ps = ypool.tile([128, QT], f32, tag="y")
            den_ps = dpool.tile([128, 8], f32, tag="den")
            den_pairs = [(kc, s)
                         for ii, (kc, _, q0e, _, _, sa) in enumerate(entries)
                         for s in range(4)
                         if sa[s] and (128 * (s + 1) > (0 if ii == 0 else q0e))]
            den_first, den_last = den_pairs[0], den_pairs[-1]

            def pv_and_den(ei, kc, q0, sub_any, p_sb):
                nc.tensor.matmul(y_ps[:, q0:QT], lhsT=qg3[:, kc, 256:384],
                                 rhs=p_sb[:, q0:QT],
                                 start=(ei == 0), stop=(ei == len(entries) - 1),
                                 skip_group_check=True)
                for s in range(4):
                    if sub_any[s] and 128 * (s + 1) > q0:
                        nc.tensor.matmul(den_ps[:, 2 * s:2 * s + 2],
                                         lhsT=p_sb[:, 128 * s:128 * (s + 1)],
                                         rhs=ones[:, 0:2],
                                         start=((kc, s) == den_first),
                                         stop=((kc, s) == den_last),
                                         skip_group_check=True)

            pend_pv = []
            for ei, (kc, mid, q0, c0, c1, sub_any) in enumerate(entries):
                if ei == 0:
                    q0 = 0  # first entry must zero the whole psum bank
                kcc = slice(128 * kc, 128 * (kc + 1))
                sp = sppool.tile([128, QT], f32, tag="s")
                nc.tensor.matmul(sp[:, q0:QT], lhsT=kT[:, kcc],
                                 rhs=qT[:, QT * j + q0:QT * (j + 1)],
                                 start=True, stop=True)
                p_sb = ppool.tile([128, QT], f16, tag="p")
                nc.scalar.activation(p_sb[:, q0:QT], sp[:, q0:QT], AF.Exp,
                                     scale=ATTN_SCALE)
                if mid is not None:
                    nc.gpsimd.tensor_tensor(p_sb[:, c0:c1], p_sb[:, c0:c1],
                                            msk[:, QT * mid + c0:QT * mid + c1],
                                            op=OP.mult)
                pend_pv.append((ei, kc, q0, sub_any, p_sb))
                if len(pend_pv) >= 3:
                    pv_and_den(*pend_pv.pop(0))
            while pend_pv:
                pv_and_den(*pend_pv.pop(0))
            den3 = den_ps[:].rearrange("p (s w) -> p s w", w=2)
            nc.vector.reciprocal(recip[:, 4 * j:4 * j + 4], den3[:, :, 0])
            nc.vector.tensor_tensor(sfin[:, 4 * j:4 * j + 4], recip[:, 4 * j:4 * j + 4],
                                    gs3[:, 4 * j:4 * j + 4, 1], op=OP.mult)
            nc.vector.tensor_copy(yT[:, qs], y_ps[:])

        def stage_d_oproj(j):
            """output projection + store for q-tile j."""
            qs = slice(QT * j, QT * (j + 1))
            ob = obpool.tile([128, 4 * DIM], f16, tag="ob")
            for s in range(4):
                b = 4 * j + s
                bc = slice(128 * b, 128 * (b + 1))
                for hh in range(2):
                    op_ps = sppool.tile([128, 512], f32, tag="s", name="op_ps")
                    nc.tensor.matmul(op_ps[:], lhsT=yT[:, bc],
                                     rhs=wo[:, 512 * hh:512 * (hh + 1)],
                                     start=True, stop=True)
                    dst = ob[:, 1024 * s + 512 * hh:1024 * s + 512 * (hh + 1)]
                    if (2 * s + hh) % 2 == 0:
                        nc.scalar.mul(dst, op_ps[:], sfin[:, b:b + 1])
                    else:
                        nc.vector.tensor_scalar_mul(dst, op_ps[:], sfin[:, b:b + 1])
                if s % 2 == 1:
                    rows = slice(QT * j + 128 * (s - 1), QT * j + 128 * (s + 1))
                    nc.sync.dma_start(
                        out_d[rows, :].rearrange("(s2 p) n -> p s2 n", p=128),
                        ob[:, 1024 * (s - 1):1024 * (s + 1)].rearrange(
                            "p (s2 n) -> p s2 n", n=DIM),
                    )

        # ---- emission schedule: attention interleaved with later-doc QKV ----
        stage_xdma(0, split=True)
        stage_xdma(1)
        stage_fve(0)
        stage_fve(1)
        load_consts2()
        stage_xdma(2)
        stage_fve(2)
        stage_xdma(3)
        stage_fve(3)
        for b in range(0, DB):
            stage_a_block(b)
        stage_b_doc(0)
        stage_c_doc(0)
        for b in range(DB, DB + 4):
            stage_a_block(b)
        stage_d_attn(0)
        for b in range(DB + 4, 2 * DB):
            stage_a_block(b)
        stage_b_doc(1)
        stage_c_doc(1)
        for b in range(2 * DB, 2 * DB + 4):
            stage_a_block(b)
        stage_d_attn(1)
        stage_d_oproj(0)
        for b in range(2 * DB + 4, 3 * DB):
            stage_a_block(b)
        stage_b_doc(2)
        stage_c_doc(2)
        for b in range(3 * DB, 3 * DB + 4):
            stage_a_block(b)
        stage_d_attn(2)
        stage_d_oproj(1)
        for b in range(3 * DB + 4, 4 * DB):
            stage_a_block(b)
        stage_b_doc(3)
        stage_c_doc(3)
        stage_d_attn(3)
        stage_d_oproj(2)
        stage_d_attn(4)
        stage_d_oproj(3)
        stage_d_attn(5)
        stage_d_oproj(4)
        stage_d_attn(6)
        stage_d_oproj(5)
        stage_d_attn(7)
        stage_d_oproj(6)
        stage_d_oproj(7)
    nc.finalize()
    return nc


_CACHE = {}


def _get_program(seqlens, bm, key_offset):
    key = (seqlens.tobytes(), int(bm), int(key_offset))
    if key not in _CACHE:
        plan, mask_arr, n_masks = _plan_attention(seqlens, bm)
        nc = _build(plan, n_masks, key_offset)
        _CACHE[key] = (nc, mask_arr, n_masks)
    return _CACHE[key]


def _make_inmaps(x, ve, qkvo_w, sa_lambdas, attn_gate_w, ve_gate_w, mask_arr):
    f1, f2 = _rope_factors()
    ones = np.ones((128, 128), F16)
    x2 = x.reshape(T, DIM)
    xT = np.ascontiguousarray(x2.T).astype(F16)
    wqkv = (sa_lambdas[0] * qkvo_w[:3 * DIM]).astype(np.float32)
    wo = (sa_lambdas[1] * qkvo_w[3 * DIM:]).astype(np.float32)
    msk16 = mask_arr.astype(F16)
    in_maps = []
    for h in range(H):
        hs = slice(h * D, (h + 1) * D)
        w_h = np.concatenate([wqkv[0 * DIM:][hs], wqkv[1 * DIM:][hs], wqkv[2 * DIM:][hs]], axis=0)
        wq386 = np.zeros((DIM, 386), np.float32)
        wq386[:, :384] = w_h.T
        wq386[:6, 384] = ve_gate_w[h, :6]
        wq386[:12, 385] = attn_gate_w[h, :12]
        vebias = np.zeros((128, 2 * NB), np.float32)
        vebias[:, 0::2] = (ve[:, :6] @ ve_gate_w[h, 6:12]).reshape(NB, 128).T
        fve = np.concatenate([f1, f2, 2.0 * ve[:, hs]], axis=1)
        in_maps.append({
            "xT": xT,
            "wqT": wq386.astype(F16),
            "vebias": vebias.astype(F16),
            "woT": np.ascontiguousarray(wo[:, hs].T).astype(F16),
            "fve": fve.astype(F16),
            "masks": msk16,
            "ones": ones,
            "gwrep": np.repeat(attn_gate_w[h, :12].astype(np.float32)[:, None], 128, 1).astype(F16),
        })
    return in_maps


def _run(inputs, trace=False):
    from concourse.bass_utils import run_bass_kernel_spmd

    x = np.asarray(inputs["x"], np.float32)
    ve = np.asarray(inputs["ve"], np.float32)
    qkvo_w = np.asarray(inputs["qkvo_w"], np.float32)
    sa_lambdas = np.asarray(inputs["sa_lambdas"], np.float32)
    attn_gate_w = np.asarray(inputs["attn_gate_w"], np.float32)
    ve_gate_w = np.asarray(inputs["ve_gate_w"], np.float32)
    seqlens = np.asarray(inputs["seqlens"])
    bm = int(np.asarray(inputs["bm_size"]))
    key_offset = int(np.asarray(inputs["key_offset"]))

    nc, mask_arr, _ = _get_program(seqlens, bm, key_offset)
    in_maps = _make_inmaps(x, ve, qkvo_w, sa_lambdas, attn_gate_w, ve_gate_w, mask_arr)
    res = run_bass_kernel_spmd(nc, in_maps, core_ids=list(range(H)), trace=trace)
    out = np.zeros((T, DIM), np.float32)
    for r in res.results:
        out += np.asarray(r["out"]).astype(np.float32)
    return out.reshape(1, T, DIM), res


def kernel(**inputs) -> np.ndarray:
    out, _ = _run(inputs, trace=False)
    return out


# revision 36
# speedup vs baseline: 1.0475x; 1.0161x over previous
"""Trainium2 Bass kernel for nn_CausalSelfAttention (sparse windowed doc-masked
attention).

Sharding: tensor-parallel over heads - 8 heads onto 8 NeuronCores, one head per
core. Each core computes its head's QKV projection (fp16 matmuls at full PE
rate), RMS-norm + RoPE + key offset, value-embedding gating, block-sparse
masked attention in transposed-score layout, attention-output gating and its
partial output projection (fp16 partials). Host sums the 8 partial [T, DIM]
outputs.

x is supplied pre-transposed (xT fp16) by the host so no PE transposes are
needed for the QKV contraction; q/k rope outputs are transposed on the DMA
engines (blocked dma_start_transpose), the key-offset shift is a pair of
SBUF->SBUF DMAs, and emission is software-pipelined across documents so the
tensor engine stays busy (and clocked at full p-state) end to end.
"""

import numpy as np
import ml_dtypes
from contextlib import ExitStack

T = 4096
DIM = 1024
H = 8
D = 128
ATTN_SCALE = 0.1
EPS = 1.1920929e-07
QT = 512
NB = T // 128      # 32 row blocks
NQ = T // QT       # 8 q-tiles
ND = 4             # docs per emission group (blocks per doc group = NB // ND)
DB = NB // ND      # 8 blocks per doc group
F16 = ml_dtypes.bfloat16  # placeholder, replaced below
F16 = np.float16


def _rope_factors():
    n = D // 4
    base = np.float32(1.0 / 1024.0)
    af = base ** np.linspace(0.0, 1.0, n, dtype=np.float32)
    af = np.repeat(af, 2)
    af = np.concatenate([af, np.zeros(D // 2, np.float32)])
    theta = np.arange(T, dtype=np.float32)[:, None] * af[None, :]
    f1 = np.cos(theta).astype(np.float32)
    f2 = np.sin(theta).astype(np.float32)
    f2[:, 1::2] *= -1.0
    return f1, f2


def _plan_attention(seqlens, bm):
    """Per q-tile chunk lists with 0/1 keep-masks (sT layout [k, q])."""
    t = np.arange(T)
    doc = np.searchsorted(seqlens, t, side="right") - 1
    doc_start = np.where(doc >= 0, seqlens[np.clip(doc, 0, len(seqlens) - 1)], 0)
    lo = np.maximum(np.maximum(t - bm, doc_start), 0)

    masks = {}          # pattern-bytes -> (mask_id, keep01 [128, QT])
    plan = []           # per q-tile: list of (kc, mid, col_lo, col_hi, sub_any)
    for j in range(NQ):
        q = np.arange(j * QT, (j + 1) * QT)
        lo_q = lo[q]
        entries = []
        for kc in range(NB):
            k = np.arange(kc * 128, kc * 128 + 128)
            M = (k[:, None] <= q[None, :]) & (k[:, None] >= lo_q[None, :])
            if not M.any():
                continue
            anyk = M.any(axis=0)
            q0 = int(np.argmax(anyk))          # first q column with any valid k
            if M[:, q0:].all():
                mid, c0, c1 = None, 0, 0
            else:
                key = M.tobytes()
                if key not in masks:
                    masks[key] = (len(masks), M.astype(np.float32))
                mid = masks[key][0]
                part = ~M[:, :].all(axis=0)
                part[:q0] = False
                c0 = int(np.argmax(part))
                c1 = int(QT - np.argmax(part[::-1]))
            sub_any = tuple(bool(M[:, s * 128:(s + 1) * 128].any()) for s in range(4))
            entries.append((kc, mid, q0, c0, c1, sub_any))
        plan.append(entries)
    n_masks = len(masks)
    if n_masks:
        arr = np.zeros((n_masks * 128, QT), np.float32)
        for _, (mid, m) in masks.items():
            arr[mid * 128:(mid + 1) * 128] = m
    else:
        arr = np.ones((128, QT), np.float32)
    return plan, arr, max(n_masks, 1)


def _build(plan, n_masks, key_offset):
    import concourse.bass as bass
    import concourse.bacc as bacc
    import concourse.mybir as mybir
    from concourse import tile

    dt = mybir.dt
    f32, f16, i32 = dt.float32, dt.float16, dt.int32
    OP = mybir.AluOpType
    AF = mybir.ActivationFunctionType

    nc = bacc.Bacc()
    xt_d = nc.dram_tensor("xT", [DIM, T], f16, kind="ExternalInput").ap()
    wq_d = nc.dram_tensor("wqT", [DIM, 386], f16, kind="ExternalInput").ap()
    vb_d = nc.dram_tensor("vebias", [128, 2 * NB], f16, kind="ExternalInput").ap()
    wo_d = nc.dram_tensor("woT", [D, DIM], f16, kind="ExternalInput").ap()
    fve_d = nc.dram_tensor("fve", [T, 384], f16, kind="ExternalInput").ap()
    msk_d = nc.dram_tensor("masks", [n_masks * 128, QT], f16, kind="ExternalInput").ap()
    on_d = nc.dram_tensor("ones", [128, 128], f16, kind="ExternalInput").ap()
    gw_d = nc.dram_tensor("gwrep", [12, 128], f16, kind="ExternalInput").ap()
    out_d = nc.dram_tensor("out", [T, DIM], f16, kind="ExternalOutput").ap()

    with ExitStack() as ctx:
        tc = ctx.enter_context(tile.TileContext(nc))
        consts = ctx.enter_context(tc.tile_pool(name="consts", bufs=1))
        state = ctx.enter_context(tc.tile_pool(name="state", bufs=1))

        wq = consts.tile([128, 8 * 386], f16)
        nc.sync.dma_start(
            wq[:].rearrange("p (c n) -> p c n", n=386),
            wq_d[:].rearrange("(c p) n -> p c n", p=128),
        )
        ones = consts.tile([128, 128], f16)
        wo = consts.tile([128, DIM], f16)
        vb = consts.tile([128, 2 * NB], f16)
        msk = consts.tile([128, n_masks * QT], f16)
        fve = state.tile([128, NB * 384], f16)

        def load_consts2():
            nc.sync.dma_start(ones[:], on_d[:])
            nc.sync.dma_start(wo[:], wo_d[:])
            nc.sync.dma_start(vb[:], vb_d[:])
            nc.sync.dma_start(
                msk[:].rearrange("p (c n) -> p c n", n=QT),
                msk_d[:].rearrange("(c p) n -> p c n", p=128),
            )

        def stage_fve(g):
            nc.sync.dma_start(
                fve[:, g * 8 * 384:(g + 1) * 8 * 384].rearrange("p (b n) -> p b n", n=384),
                fve_d[g * 1024:(g + 1) * 1024, :].rearrange("(b p) n -> p b n", p=128),
            )

        qkvg_sb = state.tile([128, NB * 386], f16)
        ro_q = state.tile([128, T], f16)
        ro_k = state.tile([128, T], f16)
        qT = state.tile([128, T], f16)
        kTt = state.tile([128, T], f16)
        kT = state.tile([128, T + 128], f16, name="kT") if key_offset else kTt
        yT = state.tile([128, T], f16)
        bn_q = state.tile([128, 2 * NB], f32)
        bn_k = state.tile([128, 2 * NB], f32)
        rs_q = state.tile([128, NB], f32)
        rs_k = state.tile([128, NB], f32)
        gates_sig = state.tile([128, 2 * NB], f32)
        recip = state.tile([128, NB], f32)
        sfin = state.tile([128, NB], f32)

        qg3 = qkvg_sb[:].rearrange("p (b c) -> p b c", c=386)
        fv3 = fve[:].rearrange("p (b c) -> p b c", c=384)
        gs3 = gates_sig[:].rearrange("p (b w) -> p b w", w=2)
        vb3 = vb[:].rearrange("p (b w) -> p b w", w=2)

        xpool = ctx.enter_context(tc.tile_pool(name="xg", bufs=2))
        scr = ctx.enter_context(tc.tile_pool(name="scr", bufs=6))
        ppool = ctx.enter_context(tc.tile_pool(name="pch", bufs=8))
        obpool = ctx.enter_context(tc.tile_pool(name="ob", bufs=2))
        qppool = ctx.enter_context(tc.tile_pool(name="qp", bufs=2, space="PSUM"))
        sppool = ctx.enter_context(tc.tile_pool(name="sp", bufs=4, space="PSUM"))
        ypool = ctx.enter_context(tc.tile_pool(name="yp", bufs=1, space="PSUM"))
        dpool = ctx.enter_context(tc.tile_pool(name="dp", bufs=1, space="PSUM"))

        xg_tiles = {}

        def stage_xdma(g, split=False):
            xg = xpool.tile([128, 8 * 1024], f16)
            halves = ((0, 256), (256, 1024)) if split else ((0, 512), (512, 1024))
            for lo, hi in halves:
                nc.sync.dma_start(
                    xg[:].rearrange("p (c t) -> p c t", t=1024)[:, :, lo:hi],
                    xt_d[:, g * 1024 + lo:g * 1024 + hi].rearrange("(c p) t -> p c t", p=128),
                )
            xg_tiles[g] = xg

        def stage_a_block(b):
            """QKV matmuls for block b + psum->sbuf copies + sumsq."""
            g, bl = b // DB, b % DB
            xg = xg_tiles[g]
            bc = slice(128 * b, 128 * (b + 1))
            qpt = qppool.tile([128, 386], f32, tag="qp", name="qpt")
            qp = qpt[:]
            for ci in range(8):
                nc.tensor.matmul(qp, lhsT=xg[:, 1024 * ci + 128 * bl:1024 * ci + 128 * (bl + 1)],
                                 rhs=wq[:, 386 * ci:386 * (ci + 1)],
                                 start=(ci == 0), stop=(ci == 7))
            nc.scalar.copy(qg3[:, b, :], qp)
            for (co, bnt) in ((0, bn_q), (128, bn_k)):
                bns = scr.tile([128, 6], f32, tag="bns")
                nc.vector.bn_stats(bns[:], qg3[:, b, co:co + 128])
                nc.vector.bn_aggr(bnt[:, 2 * b:2 * b + 2], bns[:])

        def stage_b_doc(d):
            """rsqrt + gates + v-embed gating + rope for blocks of doc-group d."""
            bs = slice(DB * d, DB * (d + 1))
            for bnt, rs in ((bn_q, rs_q), (bn_k, rs_k)):
                # E[x^2] = mean^2 + var; Quake rsqrt + 2 Newton steps
                bg = bnt[:].rearrange("p (b w) -> p b w", w=2)[:, bs, :]
                sqm = scr.tile([128, DB], f32, tag="sqm")
                nc.vector.tensor_tensor(sqm[:], bg[:, :, 0], bg[:, :, 0], op=OP.mult)
                m = scr.tile([128, DB], f32, tag="rsm")
                nc.vector.scalar_tensor_tensor(out=m[:], in0=bg[:, :, 1], scalar=EPS,
                                               in1=sqm[:], op0=OP.add, op1=OP.add)
                ii = scr.tile([128, DB], i32, tag="rsi")
                nc.vector.tensor_scalar(out=ii[:], in0=m[:].bitcast(i32), scalar1=1,
                                        scalar2=None, op0=OP.logical_shift_right)
                nc.vector.tensor_scalar(out=ii[:], in0=ii[:], scalar1=-1,
                                        scalar2=0x5F3759DF, op0=OP.mult, op1=OP.add)
                y = ii[:].bitcast(f32)
                tt = scr.tile([128, DB], f32, tag="rst")
                nc.vector.tensor_mul(tt[:], y, y)
                nc.vector.tensor_mul(tt[:], tt[:], m[:])
                nc.vector.tensor_scalar(out=tt[:], in0=tt[:], scalar1=-0.5,
                                        scalar2=1.5, op0=OP.mult, op1=OP.add)
                nc.vector.tensor_mul(y, y, tt[:])
                nc.vector.tensor_mul(tt[:], y, y)
                nc.vector.tensor_mul(tt[:], tt[:], m[:])
                nc.vector.tensor_scalar(out=tt[:], in0=tt[:], scalar1=-0.5,
                                        scalar2=1.5, op0=OP.mult, op1=OP.add)
                nc.vector.tensor_mul(rs[:, bs], y, tt[:])
            # sigmoid(x) = 1 / (1 + exp(-x)) using the Exp table + DVE recip
            gsl = slice(2 * DB * d, 2 * DB * (d + 1))
            nc.vector.tensor_tensor(qg3[:, bs, 384:386], qg3[:, bs, 384:386],
                                    vb3[:, bs, :], op=OP.add)
            ge = scr.tile([128, 2 * DB], f32, tag="ge")
            nc.scalar.activation(ge[:].rearrange("p (b w) -> p b w", w=2),
                                 qg3[:, bs, 384:386], AF.Exp, scale=-1.0)
            nc.vector.tensor_scalar(out=ge[:], in0=ge[:], scalar1=1.0,
                                    scalar2=None, op0=OP.add)
            nc.vector.reciprocal(gates_sig[:, gsl], ge[:])
            for b in range(DB * d, DB * (d + 1)):
                nc.vector.scalar_tensor_tensor(out=qg3[:, b, 256:384], in0=fv3[:, b, 256:384],
                                               scalar=gs3[:, b, 0:1], in1=qg3[:, b, 256:384],
                                               op0=OP.mult, op1=OP.add)
            # rope: two half-doc-wide sweeps per tensor, then per-block rms scale
            for half in range(2):
                blk = slice(DB * d + 4 * half, DB * d + 4 * half + 4)
                cols = slice(512 * (2 * d + half), 512 * (2 * d + half + 1))
                for (co, ro, rs) in ((0, ro_q, rs_q), (128, ro_k, rs_k)):
                    src = qg3[:, blk, co:co + 128]
                    srcf = src.rearrange("p b (n w) -> p b n w", w=2)[:, :, :, ::-1]
                    eng1 = nc.vector if d in (0, 3) else nc.gpsimd
                    t1 = scr.tile([128, 512], f16, tag="t1")
                    eng1.tensor_tensor(t1[:].rearrange("p (b n) -> p b n", n=128),
                                       fv3[:, blk, 0:128], src, op=OP.mult)
                    t2 = scr.tile([128, 512], f16, tag="t2")
                    nc.vector.tensor_tensor(
                        t2[:].rearrange("p (b n w) -> p b n w", n=64, w=2),
                        fv3[:, blk, 128:256].rearrange("p b (n w) -> p b n w", w=2),
                        srcf, op=OP.mult)
                    eng1.tensor_tensor(ro[:, cols], t1[:], t2[:], op=OP.add)
                for b in range(DB * d + 4 * half, DB * d + 4 * half + 4):
                    bc = slice(128 * b, 128 * (b + 1))
                    nc.vector.tensor_tensor(ro_q[:, bc], ro_q[:, bc],
                                            rs_q[:, b:b + 1].broadcast_to((128, 128)), op=OP.mult)
                    nc.vector.tensor_tensor(ro_k[:, bc], ro_k[:, bc],
                                            rs_k[:, b:b + 1].broadcast_to((128, 128)), op=OP.mult)

        def stage_c_doc(d):
            """transpose q/k for doc-group d (DMA engines) + key-offset shift."""
            dc = slice(1024 * d, 1024 * (d + 1))
            nc.sync.dma_start_transpose(
                qT[:, dc].rearrange("p (b t) -> p b t", t=128), ro_q[:, dc])
            nc.sync.dma_start_transpose(
                kTt[:, dc].rearrange("p (b t) -> p b t", t=128), ro_k[:, dc])
            if key_offset:
                nc.sync.dma_start(kT[0:64, dc], kTt[0:64, dc])
                if d == 0:
                    nc.sync.dma_start(kT[64:128, 1:1024], kTt[64:128, 0:1023])
                    nc.sync.dma_start(kT[64:128, 0:1], kTt[64:128, 0:1])
                else:
                    nc.sync.dma_start(kT[64:128, 1024 * d:1024 * (d + 1)],
                                      kTt[64:128, 1024 * d - 1:1024 * (d + 1) - 1])

        def stage_d_attn(j):
            """attention for q-tile j -> yT."""
            qs = slice(QT * j, QT * (j + 1))
            entries = plan[j]
            y_ps = ypool.tile([128, QT], f32, tag="y")
            den_ps = dpool.tile([128, 8], f32, tag="den")
            den_pairs = [(kc, s)
                         for ii, (kc, _, q0e, _, _, sa) in enumerate(entries)
                         for s in range(4)
                         if sa[s] and (128 * (s + 1) > (0 if ii == 0 else q0e))]
            den_first, den_last = den_pairs[0], den_pairs[-1]

            def pv_and_den(ei, kc, q0, sub_any, p_sb):
                nc.tensor.matmul(y_ps[:, q0:QT], lhsT=qg3[:, kc, 256:384],
                                 rhs=p_sb[:, q0:QT],
                                 start=(ei == 0), stop=(ei == len(entries) - 1),
                                 skip_group_check=True)
                for s in range(4):
                    if sub_any[s] and 128 * (s + 1) > q0:
                        nc.tensor.matmul(den_ps[:, 2 * s:2 * s + 2],
                                         lhsT=p_sb[:, 128 * s:128 * (s + 1)],
                                         rhs=ones[:, 0:2],
                                         start=((kc, s) == den_first),
                                         stop=((kc, s) == den_last),
                                         skip_group_check=True)

            pend_pv = []
            for ei, (kc, mid, q0, c0, c1, sub_any) in enumerate(entries):
                if ei == 0:
                    q0 = 0  # first entry must zero the whole psum bank
                kcc = slice(128 * kc, 128 * (kc + 1))
                sp = sppool.tile([128, QT], f32, tag="s")
                nc.tensor.matmul(sp[:, q0:QT], lhsT=kT[:, kcc],
                                 rhs=qT[:, QT * j + q0:QT * (j + 1)],
                                 start=True, stop=True)
                p_sb = ppool.tile([128, QT], f16, tag="p")
                nc.scalar.activation(p_sb[:, q0:QT], sp[:, q0:QT], AF.Exp,
                                     scale=ATTN_SCALE)
                if mid is not None:
                    nc.gpsimd.tensor_tensor(p_sb[:, c0:c1], p_sb[:, c0:c1],
                                            msk[:, QT * mid + c0:QT * mid + c1],
                                            op=OP.mult)
                pend_pv.append((ei, kc, q0, sub_any, p_sb))
                if len(pend_pv) >= 4:
                    pv_and_den(*pend_pv.pop(0))
            while pend_pv:
                pv_and_den(*pend_pv.pop(0))
            den3 = den_ps[:].rearrange("p (s w) -> p s w", w=2)
            nc.vector.reciprocal(recip[:, 4 * j:4 * j + 4], den3[:, :, 0])
            nc.vector.tensor_tensor(sfin[:, 4 * j:4 * j + 4], recip[:, 4 * j:4 * j + 4],
                                    gs3[:, 4 * j:4 * j + 4, 1], op=OP.mult)
            nc.vector.tensor_copy(yT[:, qs], y_ps[:])

        def stage_d_oproj(j):
            """output projection + store for q-tile j."""
            qs = slice(QT * j, QT * (j + 1))
            ob = obpool.tile([128, 4 * DIM], f16, tag="ob")
            for s in range(4):
                b = 4 * j + s
                bc = slice(128 * b, 128 * (b + 1))
                for hh in range(2):
                    op_ps = sppool.tile([128, 512], f32, tag="s", name="op_ps")
                    nc.tensor.matmul(op_ps[:], lhsT=yT[:, bc],
                                     rhs=wo[:, 512 * hh:512 * (hh + 1)],
                                     start=True, stop=True)
                    dst = ob[:, 1024 * s + 512 * hh:1024 * s + 512 * (hh + 1)]
                    if (2 * s + hh) % 2 == 0:
                        nc.scalar.mul(dst, op_ps[:], sfin[:, b:b + 1])
                    else:
                        nc.vector.tensor_scalar_mul(dst, op_ps[:], sfin[:, b:b + 1])
                if s % 2 == 1:
                    rows = slice(QT * j + 128 * (s - 1), QT * j + 128 * (s + 1))
                    nc.sync.dma_start(
                        out_d[rows, :].rearrange("(s2 p) n -> p s2 n", p=128),
                        ob[:, 1024 * (s - 1):1024 * (s + 1)].rearrange(
                            "p (s2 n) -> p s2 n", n=DIM),
                    )

        # ---- emission schedule: attention interleaved with later-doc QKV ----
        stage_xdma(0, split=True)
        stage_xdma(1)
        stage_fve(0)
        stage_fve(1)
        load_consts2()
        stage_xdma(2)
        stage_fve(2)
        stage_xdma(3)
        stage_fve(3)
        for b in range(0, DB):
            stage_a_block(b)
        stage_b_doc(0)
        stage_c_doc(0)
        for b in range(DB, DB + 4):
            stage_a_block(b)
        stage_d_attn(0)
        for b in range(DB + 4, 2 * DB):
            stage_a_block(b)
        stage_b_doc(1)
        stage_c_doc(1)
        for b in range(2 * DB, 2 * DB + 4):
            stage_a_block(b)
        stage_d_attn(1)
        stage_d_oproj(0)
        for b in range(2 * DB + 4, 3 * DB):
            stage_a_block(b)
        stage_b_doc(2)
        stage_c_doc(2)
        for b in range(3 * DB, 3 * DB + 4):
            stage_a_block(b)
        stage_d_attn(2)
        stage_d_oproj(1)
        for b in range(3 * DB + 4, 4 * DB):
            stage_a_block(b)
        stage_b_doc(3)
        stage_c_doc(3)
        stage_d_attn(3)
        stage_d_oproj(2)
        stage_d_attn(4)
        stage_d_oproj(3)
        stage_d_attn(5)
        stage_d_oproj(4)
        stage_d_attn(6)
        stage_d_oproj(5)
        stage_d_attn(7)
        stage_d_oproj(6)
        stage_d_oproj(7)
    nc.finalize()
    return nc


_CACHE = {}


def _get_program(seqlens, bm, key_offset):
    key = (seqlens.tobytes(), int(bm), int(key_offset))
    if key not in _CACHE:
        plan, mask_arr, n_masks = _plan_attention(seqlens, bm)
        nc = _build(plan, n_masks, key_offset)
        _CACHE[key] = (nc, mask_arr, n_masks)
    return _CACHE[key]


def _make_inmaps(x, ve, qkvo_w, sa_lambdas, attn_gate_w, ve_gate_w, mask_arr):
    f1, f2 = _rope_factors()
    ones = np.ones((128, 128), F16)
    x2 = x.reshape(T, DIM)
    xT = np.ascontiguousarray(x2.T).astype(F16)
    wqkv = (sa_lambdas[0] * qkvo_w[:3 * DIM]).astype(np.float32)
    wo = (sa_lambdas[1] * qkvo_w[3 * DIM:]).astype(np.float32)
    msk16 = mask_arr.astype(F16)
    in_maps = []
    for h in range(H):
        hs = slice(h * D, (h + 1) * D)
        w_h = np.concatenate([wqkv[0 * DIM:][hs], wqkv[1 * DIM:][hs], wqkv[2 * DIM:][hs]], axis=0)
        wq386 = np.zeros((DIM, 386), np.float32)
        wq386[:, :384] = w_h.T
        wq386[:6, 384] = ve_gate_w[h, :6]
        wq386[:12, 385] = attn_gate_w[h, :12]
        vebias = np.zeros((128, 2 * NB), np.float32)
        vebias[:, 0::2] = (ve[:, :6] @ ve_gate_w[h, 6:12]).reshape(NB, 128).T
        fve = np.concatenate([f1, f2, 2.0 * ve[:, hs]], axis=1)
        in_maps.append({
            "xT": xT,
            "wqT": wq386.astype(F16),
            "vebias": vebias.astype(F16),
            "woT": np.ascontiguousarray(wo[:, hs].T).astype(F16),
            "fve": fve.astype(F16),
            "masks": msk16,
            "ones": ones,
            "gwrep": np.repeat(attn_gate_w[h, :12].astype(np.float32)[:, None], 128, 1).astype(F16),
        })
    return in_maps


def _run(inputs, trace=False):
    from concourse.bass_utils import run_bass_kernel_spmd

    x = np.asarray(inputs["x"], np.float32)
    ve = np.asarray(inputs["ve"], np.float32)
    qkvo_w = np.asarray(inputs["qkvo_w"], np.float32)
    sa_lambdas = np.asarray(inputs["sa_lambdas"], np.float32)
    attn_gate_w = np.asarray(inputs["attn_gate_w"], np.float32)
    ve_gate_w = np.asarray(inputs["ve_gate_w"], np.float32)
    seqlens = np.asarray(inputs["seqlens"])
    bm = int(np.asarray(inputs["bm_size"]))
    key_offset = int(np.asarray(inputs["key_offset"]))

    nc, mask_arr, _ = _get_program(seqlens, bm, key_offset)
    in_maps = _make_inmaps(x, ve, qkvo_w, sa_lambdas, attn_gate_w, ve_gate_w, mask_arr)
    res = run_bass_kernel_spmd(nc, in_maps, core_ids=list(range(H)), trace=trace)
    out = np.zeros((T, DIM), np.float32)
    for r in res.results:
        out += np.asarray(r["out"]).astype(np.float32)
    return out.reshape(1, T, DIM), res


def kernel(**inputs) -> np.ndarray:
    out, _ = _run(inputs, trace=False)
    return out


# revision 37
# speedup vs baseline: 1.0535x; 1.0057x over previous
"""Trainium2 Bass kernel for nn_CausalSelfAttention (sparse windowed doc-masked
attention).

Sharding: tensor-parallel over heads - 8 heads onto 8 NeuronCores, one head per
core. Each core computes its head's QKV projection (fp16 matmuls at full PE
rate), RMS-norm + RoPE + key offset, value-embedding gating, block-sparse
masked attention in transposed-score layout, attention-output gating and its
partial output projection (fp16 partials). Host sums the 8 partial [T, DIM]
outputs.

x is supplied pre-transposed (xT fp16) by the host so no PE transposes are
needed for the QKV contraction; q/k rope outputs are transposed on the DMA
engines (blocked dma_start_transpose), the key-offset shift is a pair of
SBUF->SBUF DMAs, and emission is software-pipelined across documents so the
tensor engine stays busy (and clocked at full p-state) end to end.
"""

import numpy as np
import ml_dtypes
from contextlib import ExitStack

T = 4096
DIM = 1024
H = 8
D = 128
ATTN_SCALE = 0.1
EPS = 1.1920929e-07
QT = 512
NB = T // 128      # 32 row blocks
NQ = T // QT       # 8 q-tiles
ND = 4             # docs per emission group (blocks per doc group = NB // ND)
DB = NB // ND      # 8 blocks per doc group
F16 = ml_dtypes.bfloat16  # placeholder, replaced below
F16 = np.float16


def _rope_factors():
    n = D // 4
    base = np.float32(1.0 / 1024.0)
    af = base ** np.linspace(0.0, 1.0, n, dtype=np.float32)
    af = np.repeat(af, 2)
    af = np.concatenate([af, np.zeros(D // 2, np.float32)])
    theta = np.arange(T, dtype=np.float32)[:, None] * af[None, :]
    f1 = np.cos(theta).astype(np.float32)
    f2 = np.sin(theta).astype(np.float32)
    f2[:, 1::2] *= -1.0
    return f1, f2


def _plan_attention(seqlens, bm):
    """Per q-tile chunk lists with 0/1 keep-masks (sT layout [k, q])."""
    t = np.arange(T)
    doc = np.searchsorted(seqlens, t, side="right") - 1
    doc_start = np.where(doc >= 0, seqlens[np.clip(doc, 0, len(seqlens) - 1)], 0)
    lo = np.maximum(np.maximum(t - bm, doc_start), 0)

    masks = {}          # pattern-bytes -> (mask_id, keep01 [128, QT])
    plan = []           # per q-tile: list of (kc, mid, col_lo, col_hi, sub_any)
    for j in range(NQ):
        q = np.arange(j * QT, (j + 1) * QT)
        lo_q = lo[q]
        entries = []
        for kc in range(NB):
            k = np.arange(kc * 128, kc * 128 + 128)
            M = (k[:, None] <= q[None, :]) & (k[:, None] >= lo_q[None, :])
            if not M.any():
                continue
            anyk = M.any(axis=0)
            q0 = int(np.argmax(anyk))          # first q column with any valid k
            if M[:, q0:].all():
                mid, c0, c1 = None, 0, 0
            else:
                key = M.tobytes()
                if key not in masks:
                    masks[key] = (len(masks), M.astype(np.float32))
                mid = masks[key][0]
                part = ~M[:, :].all(axis=0)
                part[:q0] = False
                c0 = int(np.argmax(part))
                c1 = int(QT - np.argmax(part[::-1]))
            sub_any = tuple(bool(M[:, s * 128:(s + 1) * 128].any()) for s in range(4))
            entries.append((kc, mid, q0, c0, c1, sub_any))
        plan.append(entries)
    n_masks = len(masks)
    if n_masks:
        arr = np.zeros((n_masks * 128, QT), np.float32)
        for _, (mid, m) in masks.items():
            arr[mid * 128:(mid + 1) * 128] = m
    else:
        arr = np.ones((128, QT), np.float32)
    return plan, arr, max(n_masks, 1)


def _build(plan, n_masks, key_offset):
    import concourse.bass as bass
    import concourse.bacc as bacc
    import concourse.mybir as mybir
    from concourse import tile

    dt = mybir.dt
    f32, f16, i32 = dt.float32, dt.float16, dt.int32
    OP = mybir.AluOpType
    AF = mybir.ActivationFunctionType

    nc = bacc.Bacc()
    xt_d = nc.dram_tensor("xT", [DIM, T], f16, kind="ExternalInput").ap()
    wq_d = nc.dram_tensor("wqT", [DIM, 386], f16, kind="ExternalInput").ap()
    vb_d = nc.dram_tensor("vebias", [128, 2 * NB], f16, kind="ExternalInput").ap()
    wo_d = nc.dram_tensor("woT", [D, DIM], f16, kind="ExternalInput").ap()
    fve_d = nc.dram_tensor("fve", [T, 384], f16, kind="ExternalInput").ap()
    msk_d = nc.dram_tensor("masks", [n_masks * 128, QT], f16, kind="ExternalInput").ap()
    on_d = nc.dram_tensor("ones", [128, 128], f16, kind="ExternalInput").ap()
    gw_d = nc.dram_tensor("gwrep", [12, 128], f16, kind="ExternalInput").ap()
    out_d = nc.dram_tensor("out", [T, DIM], f16, kind="ExternalOutput").ap()

    with ExitStack() as ctx:
        tc = ctx.enter_context(tile.TileContext(nc))
        consts = ctx.enter_context(tc.tile_pool(name="consts", bufs=1))
        state = ctx.enter_context(tc.tile_pool(name="state", bufs=1))

        wq = consts.tile([128, 8 * 386], f16)
        nc.sync.dma_start(
            wq[:].rearrange("p (c n) -> p c n", n=386),
            wq_d[:].rearrange("(c p) n -> p c n", p=128),
        )
        ones = consts.tile([128, 128], f16)
        wo = consts.tile([128, DIM], f16)
        vb = consts.tile([128, 2 * NB], f16)
        msk = consts.tile([128, n_masks * QT], f16)
        fve = state.tile([128, NB * 384], f16)

        def load_consts2():
            nc.sync.dma_start(ones[:], on_d[:])
            nc.sync.dma_start(wo[:], wo_d[:])
            nc.sync.dma_start(vb[:], vb_d[:])
            nc.sync.dma_start(
                msk[:].rearrange("p (c n) -> p c n", n=QT),
                msk_d[:].rearrange("(c p) n -> p c n", p=128),
            )

        def stage_fve(g):
            nc.sync.dma_start(
                fve[:, g * 8 * 384:(g + 1) * 8 * 384].rearrange("p (b n) -> p b n", n=384),
                fve_d[g * 1024:(g + 1) * 1024, :].rearrange("(b p) n -> p b n", p=128),
            )

        qkvg_sb = state.tile([128, NB * 386], f16)
        ro_q = state.tile([128, T], f16)
        ro_k = state.tile([128, T], f16)
        qT = state.tile([128, T], f16)
        kTt = state.tile([128, T], f16)
        kT = state.tile([128, T + 128], f16, name="kT") if key_offset else kTt
        yT = state.tile([128, T], f16)
        bn_q = state.tile([128, 2 * NB], f32)
        bn_k = state.tile([128, 2 * NB], f32)
        rs_q = state.tile([128, NB], f32)
        rs_k = state.tile([128, NB], f32)
        gates_sig = state.tile([128, 2 * NB], f32)
        recip = state.tile([128, NB], f32)
        sfin = state.tile([128, NB], f32)

        qg3 = qkvg_sb[:].rearrange("p (b c) -> p b c", c=386)
        fv3 = fve[:].rearrange("p (b c) -> p b c", c=384)
        gs3 = gates_sig[:].rearrange("p (b w) -> p b w", w=2)
        vb3 = vb[:].rearrange("p (b w) -> p b w", w=2)

        xpool = ctx.enter_context(tc.tile_pool(name="xg", bufs=2))
        scr = ctx.enter_context(tc.tile_pool(name="scr", bufs=8))
        ppool = ctx.enter_context(tc.tile_pool(name="pch", bufs=10))
        obpool = ctx.enter_context(tc.tile_pool(name="ob", bufs=2))
        qppool = ctx.enter_context(tc.tile_pool(name="qp", bufs=2, space="PSUM"))
        sppool = ctx.enter_context(tc.tile_pool(name="sp", bufs=4, space="PSUM"))
        ypool = ctx.enter_context(tc.tile_pool(name="yp", bufs=1, space="PSUM"))
        dpool = ctx.enter_context(tc.tile_pool(name="dp", bufs=1, space="PSUM"))

        xg_tiles = {}

        def stage_xdma(g, split=False):
            xg = xpool.tile([128, 8 * 1024], f16)
            halves = ((0, 256), (256, 1024)) if split else ((0, 512), (512, 1024))
            for lo, hi in halves:
                nc.sync.dma_start(
                    xg[:].rearrange("p (c t) -> p c t", t=1024)[:, :, lo:hi],
                    xt_d[:, g * 1024 + lo:g * 1024 + hi].rearrange("(c p) t -> p c t", p=128),
                )
            xg_tiles[g] = xg

        def stage_a_block(b):
            """QKV matmuls for block b + psum->sbuf copies + sumsq."""
            g, bl = b // DB, b % DB
            xg = xg_tiles[g]
            bc = slice(128 * b, 128 * (b + 1))
            qpt = qppool.tile([128, 386], f32, tag="qp", name="qpt")
            qp = qpt[:]
            for ci in range(8):
                nc.tensor.matmul(qp, lhsT=xg[:, 1024 * ci + 128 * bl:1024 * ci + 128 * (bl + 1)],
                                 rhs=wq[:, 386 * ci:386 * (ci + 1)],
                                 start=(ci == 0), stop=(ci == 7))
            nc.scalar.copy(qg3[:, b, :], qp)
            for (co, bnt) in ((0, bn_q), (128, bn_k)):
                bns = scr.tile([128, 6], f32, tag="bns")
                nc.vector.bn_stats(bns[:], qg3[:, b, co:co + 128])
                nc.vector.bn_aggr(bnt[:, 2 * b:2 * b + 2], bns[:])

        def stage_b_doc(d):
            """rsqrt + gates + v-embed gating + rope for blocks of doc-group d."""
            bs = slice(DB * d, DB * (d + 1))
            for bnt, rs in ((bn_q, rs_q), (bn_k, rs_k)):
                # E[x^2] = mean^2 + var; Quake rsqrt + 2 Newton steps
                bg = bnt[:].rearrange("p (b w) -> p b w", w=2)[:, bs, :]
                sqm = scr.tile([128, DB], f32, tag="sqm")
                nc.vector.tensor_tensor(sqm[:], bg[:, :, 0], bg[:, :, 0], op=OP.mult)
                m = scr.tile([128, DB], f32, tag="rsm")
                nc.vector.scalar_tensor_tensor(out=m[:], in0=bg[:, :, 1], scalar=EPS,
                                               in1=sqm[:], op0=OP.add, op1=OP.add)
                ii = scr.tile([128, DB], i32, tag="rsi")
                nc.vector.tensor_scalar(out=ii[:], in0=m[:].bitcast(i32), scalar1=1,
                                        scalar2=None, op0=OP.logical_shift_right)
                nc.vector.tensor_scalar(out=ii[:], in0=ii[:], scalar1=-1,
                                        scalar2=0x5F3759DF, op0=OP.mult, op1=OP.add)
                y = ii[:].bitcast(f32)
                tt = scr.tile([128, DB], f32, tag="rst")
                nc.vector.tensor_mul(tt[:], y, y)
                nc.vector.tensor_mul(tt[:], tt[:], m[:])
                nc.vector.tensor_scalar(out=tt[:], in0=tt[:], scalar1=-0.5,
                                        scalar2=1.5, op0=OP.mult, op1=OP.add)
                nc.vector.tensor_mul(y, y, tt[:])
                nc.vector.tensor_mul(tt[:], y, y)
                nc.vector.tensor_mul(tt[:], tt[:], m[:])
                nc.vector.tensor_scalar(out=tt[:], in0=tt[:], scalar1=-0.5,
                                        scalar2=1.5, op0=OP.mult, op1=OP.add)
                nc.vector.tensor_mul(rs[:, bs], y, tt[:])
            # sigmoid(x) = 1 / (1 + exp(-x)) using the Exp table + DVE recip
            gsl = slice(2 * DB * d, 2 * DB * (d + 1))
            nc.vector.tensor_tensor(qg3[:, bs, 384:386], qg3[:, bs, 384:386],
                                    vb3[:, bs, :], op=OP.add)
            ge = scr.tile([128, 2 * DB], f32, tag="ge")
            nc.scalar.activation(ge[:].rearrange("p (b w) -> p b w", w=2),
                                 qg3[:, bs, 384:386], AF.Exp, scale=-1.0)
            nc.vector.tensor_scalar(out=ge[:], in0=ge[:], scalar1=1.0,
                                    scalar2=None, op0=OP.add)
            nc.vector.reciprocal(gates_sig[:, gsl], ge[:])
            for b in range(DB * d, DB * (d + 1)):
                nc.vector.scalar_tensor_tensor(out=qg3[:, b, 256:384], in0=fv3[:, b, 256:384],
                                               scalar=gs3[:, b, 0:1], in1=qg3[:, b, 256:384],
                                               op0=OP.mult, op1=OP.add)
            # rope: two half-doc-wide sweeps per tensor, then per-block rms scale
            for half in range(2):
                blk = slice(DB * d + 4 * half, DB * d + 4 * half + 4)
                cols = slice(512 * (2 * d + half), 512 * (2 * d + half + 1))
                for (co, ro, rs) in ((0, ro_q, rs_q), (128, ro_k, rs_k)):
                    src = qg3[:, blk, co:co + 128]
                    srcf = src.rearrange("p b (n w) -> p b n w", w=2)[:, :, :, ::-1]
                    eng1 = nc.vector if d in (0, 3) else nc.gpsimd
                    t1 = scr.tile([128, 512], f16, tag="t1")
                    eng1.tensor_tensor(t1[:].rearrange("p (b n) -> p b n", n=128),
                                       fv3[:, blk, 0:128], src, op=OP.mult)
                    t2 = scr.tile([128, 512], f16, tag="t2")
                    nc.vector.tensor_tensor(
                        t2[:].rearrange("p (b n w) -> p b n w", n=64, w=2),
                        fv3[:, blk, 128:256].rearrange("p b (n w) -> p b n w", w=2),
                        srcf, op=OP.mult)
                    eng1.tensor_tensor(ro[:, cols], t1[:], t2[:], op=OP.add)
                for b in range(DB * d + 4 * half, DB * d + 4 * half + 4):
                    bc = slice(128 * b, 128 * (b + 1))
                    nc.vector.tensor_tensor(ro_q[:, bc], ro_q[:, bc],
                                            rs_q[:, b:b + 1].broadcast_to((128, 128)), op=OP.mult)
                    nc.vector.tensor_tensor(ro_k[:, bc], ro_k[:, bc],
                                            rs_k[:, b:b + 1].broadcast_to((128, 128)), op=OP.mult)

        def stage_c_doc(d):
            """transpose q/k for doc-group d (DMA engines) + key-offset shift."""
            dc = slice(1024 * d, 1024 * (d + 1))
            nc.sync.dma_start_transpose(
                qT[:, dc].rearrange("p (b t) -> p b t", t=128), ro_q[:, dc])
            nc.sync.dma_start_transpose(
                kTt[:, dc].rearrange("p (b t) -> p b t", t=128), ro_k[:, dc])
            if key_offset:
                nc.sync.dma_start(kT[0:64, dc], kTt[0:64, dc])
                if d == 0:
                    nc.sync.dma_start(kT[64:128, 1:1024], kTt[64:128, 0:1023])
                    nc.sync.dma_start(kT[64:128, 0:1], kTt[64:128, 0:1])
                else:
                    nc.sync.dma_start(kT[64:128, 1024 * d:1024 * (d + 1)],
                                      kTt[64:128, 1024 * d - 1:1024 * (d + 1) - 1])

        def stage_d_attn(j):
            """attention for q-tile j -> yT."""
            qs = slice(QT * j, QT * (j + 1))
            entries = plan[j]
            y_ps = ypool.tile([128, QT], f32, tag="y")
            den_ps = dpool.tile([128, 8], f32, tag="den")
            den_pairs = [(kc, s)
                         for ii, (kc, _, q0e, _, _, sa) in enumerate(entries)
                         for s in range(4)
                         if sa[s] and (128 * (s + 1) > (0 if ii == 0 else q0e))]
            den_first, den_last = den_pairs[0], den_pairs[-1]

            def pv_and_den(ei, kc, q0, sub_any, p_sb):
                nc.tensor.matmul(y_ps[:, q0:QT], lhsT=qg3[:, kc, 256:384],
                                 rhs=p_sb[:, q0:QT],
                                 start=(ei == 0), stop=(ei == len(entries) - 1),
                                 skip_group_check=True)
                for s in range(4):
                    if sub_any[s] and 128 * (s + 1) > q0:
                        nc.tensor.matmul(den_ps[:, 2 * s:2 * s + 2],
                                         lhsT=p_sb[:, 128 * s:128 * (s + 1)],
                                         rhs=ones[:, 0:2],
                                         start=((kc, s) == den_first),
                                         stop=((kc, s) == den_last),
                                         skip_group_check=True)

            pend_pv = []
            for ei, (kc, mid, q0, c0, c1, sub_any) in enumerate(entries):
                if ei == 0:
                    q0 = 0  # first entry must zero the whole psum bank
                kcc = slice(128 * kc, 128 * (kc + 1))
                sp = sppool.tile([128, QT], f32, tag="s")
                nc.tensor.matmul(sp[:, q0:QT], lhsT=kT[:, kcc],
                                 rhs=qT[:, QT * j + q0:QT * (j + 1)],
                                 start=True, stop=True)
                p_sb = ppool.tile([128, QT], f16, tag="p")
                nc.scalar.activation(p_sb[:, q0:QT], sp[:, q0:QT], AF.Exp,
                                     scale=ATTN_SCALE)
                if mid is not None:
                    nc.gpsimd.tensor_tensor(p_sb[:, c0:c1], p_sb[:, c0:c1],
                                            msk[:, QT * mid + c0:QT * mid + c1],
                                            op=OP.mult)
                pend_pv.append((ei, kc, q0, sub_any, p_sb))
                if len(pend_pv) >= 5:
                    pv_and_den(*pend_pv.pop(0))
            while pend_pv:
                pv_and_den(*pend_pv.pop(0))
            den3 = den_ps[:].rearrange("p (s w) -> p s w", w=2)
            nc.vector.reciprocal(recip[:, 4 * j:4 * j + 4], den3[:, :, 0])
            nc.vector.tensor_tensor(sfin[:, 4 * j:4 * j + 4], recip[:, 4 * j:4 * j + 4],
                                    gs3[:, 4 * j:4 * j + 4, 1], op=OP.mult)
            nc.vector.tensor_copy(yT[:, qs], y_ps[:])

        def stage_d_oproj(j):
            """output projection + store for q-tile j."""
            qs = slice(QT * j, QT * (j + 1))
            ob = obpool.tile([128, 4 * DIM], f16, tag="ob")
            for s in range(4):
                b = 4 * j + s
                bc = slice(128 * b, 128 * (b + 1))
                for hh in range(2):
                    op_ps = sppool.tile([128, 512], f32, tag="s", name="op_ps")
                    nc.tensor.matmul(op_ps[:], lhsT=yT[:, bc],
                                     rhs=wo[:, 512 * hh:512 * (hh + 1)],
                                     start=True, stop=True)
                    dst = ob[:, 1024 * s + 512 * hh:1024 * s + 512 * (hh + 1)]
                    if (2 * s + hh) % 2 == 0:
                        nc.scalar.mul(dst, op_ps[:], sfin[:, b:b + 1])
                    else:
                        nc.vector.tensor_scalar_mul(dst, op_ps[:], sfin[:, b:b + 1])
                if s % 2 == 1:
                    rows = slice(QT * j + 128 * (s - 1), QT * j + 128 * (s + 1))
                    nc.sync.dma_start(
                        out_d[rows, :].rearrange("(s2 p) n -> p s2 n", p=128),
                        ob[:, 1024 * (s - 1):1024 * (s + 1)].rearrange(
                            "p (s2 n) -> p s2 n", n=DIM),
                    )

        # ---- emission schedule: attention interleaved with later-doc QKV ----
        stage_xdma(0, split=True)
        stage_xdma(1)
        stage_fve(0)
        stage_fve(1)
        load_consts2()
        stage_xdma(2)
        stage_fve(2)
        stage_xdma(3)
        stage_fve(3)
        for b in range(0, DB):
            stage_a_block(b)
        stage_b_doc(0)
        stage_c_doc(0)
        for b in range(DB, DB + 4):
            stage_a_block(b)
        stage_d_attn(0)
        for b in range(DB + 4, 2 * DB):
            stage_a_block(b)
        stage_b_doc(1)
        stage_c_doc(1)
        for b in range(2 * DB, 2 * DB + 4):
            stage_a_block(b)
        stage_d_attn(1)
        stage_d_oproj(0)
        for b in range(2 * DB + 4, 3 * DB):
            stage_a_block(b)
        stage_b_doc(2)
        stage_c_doc(2)
        for b in range(3 * DB, 3 * DB + 4):
            stage_a_block(b)
        stage_d_attn(2)
        stage_d_oproj(1)
        for b in range(3 * DB + 4, 4 * DB):
            stage_a_block(b)
        stage_b_doc(3)
        stage_c_doc(3)
        stage_d_attn(3)
        stage_d_oproj(2)
        stage_d_attn(4)
        stage_d_oproj(3)
        stage_d_attn(5)
        stage_d_oproj(4)
        stage_d_attn(6)
        stage_d_oproj(5)
        stage_d_attn(7)
        stage_d_oproj(6)
        stage_d_oproj(7)
    nc.finalize()
    return nc


_CACHE = {}


def _get_program(seqlens, bm, key_offset):
    key = (seqlens.tobytes(), int(bm), int(key_offset))
    if key not in _CACHE:
        plan, mask_arr, n_masks = _plan_attention(seqlens, bm)
        nc = _build(plan, n_masks, key_offset)
        _CACHE[key] = (nc, mask_arr, n_masks)
    return _CACHE[key]


def _make_inmaps(x, ve, qkvo_w, sa_lambdas, attn_gate_w, ve_gate_w, mask_arr):
    f1, f2 = _rope_factors()
    ones = np.ones((128, 128), F16)
    x2 = x.reshape(T, DIM)
    xT = np.ascontiguousarray(x2.T).astype(F16)
    wqkv = (sa_lambdas[0] * qkvo_w[:3 * DIM]).astype(np.float32)
    wo = (sa_lambdas[1] * qkvo_w[3 * DIM:]).astype(np.float32)
    msk16 = mask_arr.astype(F16)
    in_maps = []
    for h in range(H):
        hs = slice(h * D, (h + 1) * D)
        w_h = np.concatenate([wqkv[0 * DIM:][hs], wqkv[1 * DIM:][hs], wqkv[2 * DIM:][hs]], axis=0)
        wq386 = np.zeros((DIM, 386), np.float32)
        wq386[:, :384] = w_h.T
        wq386[:6, 384] = ve_gate_w[h, :6]
        wq386[:12, 385] = attn_gate_w[h, :12]
        vebias = np.zeros((128, 2 * NB), np.float32)
        vebias[:, 0::2] = (ve[:, :6] @ ve_gate_w[h, 6:12]).reshape(NB, 128).T
        fve = np.concatenate([f1, f2, 2.0 * ve[:, hs]], axis=1)
        in_maps.append({
            "xT": xT,
            "wqT": wq386.astype(F16),
            "vebias": vebias.astype(F16),
            "woT": np.ascontiguousarray(wo[:, hs].T).astype(F16),
            "fve": fve.astype(F16),
            "masks": msk16,
            "ones": ones,
            "gwrep": np.repeat(attn_gate_w[h, :12].astype(np.float32)[:, None], 128, 1).astype(F16),
        })
    return in_maps


def _run(inputs, trace=False):
    from concourse.bass_utils import run_bass_kernel_spmd

    x = np.asarray(inputs["x"], np.float32)
    ve = np.asarray(inputs["ve"], np.float32)
    qkvo_w = np.asarray(inputs["qkvo_w"], np.float32)
    sa_lambdas = np.asarray(inputs["sa_lambdas"], np.float32)
    attn_gate_w = np.asarray(inputs["attn_gate_w"], np.float32)
    ve_gate_w = np.asarray(inputs["ve_gate_w"], np.float32)
    seqlens = np.asarray(inputs["seqlens"])
    bm = int(np.asarray(inputs["bm_size"]))
    key_offset = int(np.asarray(inputs["key_offset"]))

    nc, mask_arr, _ = _get_program(seqlens, bm, key_offset)
    in_maps = _make_inmaps(x, ve, qkvo_w, sa_lambdas, attn_gate_w, ve_gate_w, mask_arr)
    res = run_bass_kernel_spmd(nc, in_maps, core_ids=list(range(H)), trace=trace)
    out = np.zeros((T, DIM), np.float32)
    for r in res.results:
        out += np.asarray(r["out"]).astype(np.float32)
    return out.reshape(1, T, DIM), res


def kernel(**inputs) -> np.ndarray:
    out, _ = _run(inputs, trace=False)
    return out
